# revision 4
# baseline (speedup 1.0000x reference)
"""Multi-layer tanh RNN on 8 Trainium2 NeuronCores.

Strategy (sequence-split x batch-split, 32 batch rows/core, plain bf16):
- 8 cores = 2 sequence-halves x 4 batch-quarters. Cores 0-3 run t in
  [0, 272) exactly (h starts at 0); cores 4-7 run t in [240, 512) where
  the first 32 steps are warmup (the tanh RNN is contractive, so the
  hidden state reconverges to the true orbit well below bf16 noise).
  Every core runs the same 272-step program -- pure SPMD, no collectives.
- Wavefront over the 4 layers: at wavefront s, layer j processes t = s - j.
  The 4 (layer, t) units run CONCURRENTLY in the PE array via 4-way column
  tiling (tile_position=(0, 32j)); each stripe's stationary is the full 32
  batch rows in single bf16 (no hi/lo compensation -- bf16 weight rounding
  dominates the error budget anyway), so every PE column does useful work.
- Per wavefront: PE streams all weights (K = 2048 per layer, bf16), the
  batch-major preact gets bias (DVE) + tanh (scalar) and is cast to bf16,
  then a bf16 identity-transpose on the PE (1 cycle/row) restores H-major
  for the next step's stationaries. Output leaves batch-major as bf16.
- Weights (W_ih^T and W_hh^T concatenated per layer) are bf16, SBUF-resident.

Host side: builds per-core x windows ([T,H,B_c] bf16), re-assembles the
output from the non-warmup slots.
"""
import numpy as np
import ml_dtypes

import concourse.bass as bass
import concourse.bacc as bacc
import concourse.mybir as mybir
from concourse import tile
from concourse.bass_utils import run_bass_kernel_spmd

F32 = mybir.dt.float32
BF16 = mybir.dt.bfloat16

SEQ, BATCH, HID, LAYERS = 512, 128, 1024, 4
NCORES = 8
WARM = 16                     # warmup steps for the second-half cores
TW = (SEQ + WARM) // 2        # 272-step window per core
BC = 32                       # batch rows per core (4 quarters)
CH = HID // 128               # 8 H-chunks
KT = 2 * HID // 128           # 16 K-tiles (x-part 0..7, h-part 8..15)
XPAD = 4                      # zero-padded extra timesteps for x prefetch


def build_kernel(repeat: int = 1):
    nc = bacc.Bacc("TRN2", target_bir_lowering=False, debug=False)

    d_x = nc.dram_tensor("x_w", (TW + XPAD, HID, BC), BF16,
                         kind="ExternalInput").ap()
    d_w = nc.dram_tensor("w_hi", (LAYERS, 2 * HID, HID), BF16,
                         kind="ExternalInput").ap()
    d_bias = nc.dram_tensor("bias_bm", (128, HID), F32,
                            kind="ExternalInput").ap()
    d_ident = nc.dram_tensor("ident", (128, 128), BF16,
                             kind="ExternalInput").ap()
    d_out = nc.dram_tensor("out_bm", (TW, BC, HID), BF16,
                           kind="ExternalOutput").ap()

    # DRAM views tiled for DMA: [T, H, B] -> [T, chunk, part, B]
    v_x = d_x.rearrange("t (c p) b -> t c p b", p=128)
    v_w = d_w.rearrange("l (k p) n -> l k p n", p=128)

    with tile.TileContext(nc) as tc:
        with (
            tc.tile_pool(name="sbw", bufs=1) as pw,
            tc.tile_pool(name="sbs", bufs=1) as ps,
            tc.tile_pool(name="psA", bufs=1, space="PSUM") as ppa,
            tc.tile_pool(name="psB", bufs=1, space="PSUM") as ppb,
        ):
            # weights: [128, layer, ktile, H]  (128 KB/partition)
            w_sb = pw.tile([128, LAYERS, KT, HID], BF16)
            # h stationaries, H-major: [128, parity, chunk*layer*32]
            hbuf = ps.tile([128, 2, CH * 4 * BC], BF16)
            # x stationaries: [128, parity, chunk, 32]
            xbuf = ps.tile([128, 2, CH, BC], BF16)
            # biased preact staging (batch-major, f32)
            stg = ps.tile([128, 2, HID], F32)
            # tanh output bf16, batch-major (transpose source + out DMA src)
            hbm = ps.tile([128, 2, HID], BF16)
            bias_sb = ps.tile([128, HID], F32)
            ident_sb = ps.tile([128, 128], BF16)

            psum_mm = [ppa.tile([128, HID], F32, tag=f"pmm{i}", name=f"pmm{i}")
                       for i in range(2)]
            psumT = [ppb.tile([128, HID], BF16, tag=f"pT{i}",
                              name=f"pT{i}") for i in range(2)]

            # ---- init ----
            for l in range(LAYERS):
                nc.sync.dma_start(out=w_sb[:, l], in_=v_w[l].transpose([1, 0, 2]))
            nc.sync.dma_start(out=bias_sb[:], in_=d_bias)
            nc.sync.dma_start(out=ident_sb[:], in_=d_ident)
            nc.vector.memset(hbuf[:], 0.0)
            nc.vector.memset(hbm[:], 0.0)
            nc.vector.memset(stg[:], 0.0)

            def tslice(v, t):
                a = v[t]
                if a.ndim == 4:
                    a = a.squeeze(0)
                return a.transpose([1, 0, 2])

            def dma_x(t_idx, parity):
                nc.sync.dma_start(out=xbuf[:, parity], in_=tslice(v_x, t_idx))

            def stationary(g, k, p):
                """lhsT [128, 32] for unit g, K-tile k, current parity p."""
                if k < CH:  # input part: x for layer 0, h_{g-1} otherwise
                    if g == 0:
                        return xbuf[:, p, k, :]
                    return hbuf[:, 1 - p, 128 * k + BC * (g - 1):
                                128 * k + BC * g]
                return hbuf[:, 1 - p, 128 * (k - CH) + BC * g:
                            128 * (k - CH) + BC * (g + 1)]

            def wavefront(p, units, out_t=None, x_t=None, prefetch_t=None,
                          out_units=None):
                """Emit one wavefront.

                p: parity (0/1). units: active unit (=layer) list.
                out_t: DRAM index expr for the unit-3 output DMA (or None).
                x_t: synchronous x load for this wavefront (prologue only).
                prefetch_t: x load for wavefront +2 (steady state).
                out_units: units whose postproc should write hbuf (defaults
                  to `units`).
                """
                if out_units is None:
                    out_units = units
                if x_t is not None:
                    dma_x(x_t, p)
                pm = psum_mm[p]
                pt = psumT[p]
                full = len(units) == 4
                # matmul streams: halves outer, K-tiles, groups inner.
                # Both halves' streams are emitted before any postproc so the
                # PE queue never stalls on scalar/vector work mid-wavefront.
                for h in range(2):
                    for k in range(KT):
                        for g in units:
                            nc.tensor.matmul(
                                pm[32 * g:32 * (g + 1),
                                   512 * h:512 * (h + 1)],
                                stationary(g, k, p),
                                w_sb[:, g, k, 512 * h:512 * (h + 1)],
                                start=(k == 0), stop=(k == KT - 1),
                                tile_position=(0, 32 * g),
                            )
                for h in range(2):
                    # postproc for this half: bias+tanh at half granularity,
                    # transposes per 128-chunk (PE stationary width limit)
                    lo_h, hi_h = 512 * h, 512 * (h + 1)
                    if full:
                        nc.vector.tensor_add(stg[:, p, lo_h:hi_h],
                                             pm[:, lo_h:hi_h],
                                             bias_sb[:, lo_h:hi_h])
                        nc.scalar.activation(
                            hbm[:, p, lo_h:hi_h],
                            stg[:, p, lo_h:hi_h],
                            mybir.ActivationFunctionType.Tanh)
                    else:
                        for g in units:
                            r0, r1 = 32 * g, 32 * (g + 1)
                            nc.vector.tensor_add(
                                stg[r0:r1, p, lo_h:hi_h],
                                pm[r0:r1, lo_h:hi_h],
                                bias_sb[r0:r1, lo_h:hi_h])
                            nc.scalar.activation(
                                hbm[r0:r1, p, lo_h:hi_h],
                                stg[r0:r1, p, lo_h:hi_h],
                                mybir.ActivationFunctionType.Tanh)
                    for c in range(4 * h, 4 * h + 4):
                        lo_c, hi_c = 128 * c, 128 * (c + 1)
                        # bf16 identity transpose back to H-major
                        nc.tensor.matmul(pt[:, lo_c:hi_c],
                                         hbm[:, p, lo_c:hi_c], ident_sb[:],
                                         is_transpose=True,
                                         start=True, stop=True)
                    if full:
                        nc.vector.tensor_copy(hbuf[:, p, lo_h:hi_h],
                                              pt[:, lo_h:hi_h])
                    else:
                        for c in range(4 * h, 4 * h + 4):
                            for g in out_units:
                                nc.vector.tensor_copy(
                                    hbuf[:, p, 128 * c + BC * g:
                                         128 * c + BC * (g + 1)],
                                    pt[:, 128 * c + 32 * g:128 * c + 32 * (g + 1)])
                if out_t is not None:
                    o = d_out[out_t]
                    if o.ndim == 3:
                        o = o.squeeze(0)
                    nc.sync.dma_start(out=o, in_=hbm[96:128, p, :])
                if prefetch_t is not None:
                    dma_x(prefetch_t, p)

            import contextlib

            rep_ctx = (tc.For_i(0, repeat, 1) if repeat > 1
                       else contextlib.nullcontext())
            with rep_ctx:
                if repeat > 1:
                    nc.vector.memset(hbuf[:], 0.0)
                # prologue s = 0..3
                wavefront(0, [0], x_t=0)
                wavefront(1, [0, 1], x_t=1)
                wavefront(0, [0, 1, 2], x_t=2)
                wavefront(1, [0, 1, 2, 3], x_t=3, out_t=0)
                dma_x(4, 0)
                dma_x(5, 1)
                # steady state s = 4..TW-1 (parity-unrolled by 4)
                with tc.For_i(4, TW, 4, hint_engines=(mybir.EngineType.PE,),
                              name="wf") as s:
                    wavefront(0, [0, 1, 2, 3], out_t=bass.ds(s - 3, 1),
                              prefetch_t=bass.ds(s + 2, 1))
                    wavefront(1, [0, 1, 2, 3], out_t=bass.ds(s - 2, 1),
                              prefetch_t=bass.ds(s + 3, 1))
                    wavefront(0, [0, 1, 2, 3], out_t=bass.ds(s - 1, 1),
                              prefetch_t=bass.ds(s + 4, 1))
                    wavefront(1, [0, 1, 2, 3], out_t=bass.ds(s, 1),
                              prefetch_t=bass.ds(s + 5, 1))
                # epilogue s = TW..TW+2
                wavefront(0, [1, 2, 3], out_t=TW - 3)
                wavefront(1, [2, 3], out_t=TW - 2)
                wavefront(0, [3], out_t=TW - 1)

    nc.compile()
    return nc


def _prep_inputs(x, W_ih, W_hh, b_ih, b_hh):
    """Host-side prep shared across cores + per-core shards."""
    bf16 = ml_dtypes.bfloat16
    # weights: concat [W_ih^T; W_hh^T] per layer -> [L, 2H, H] bf16
    w = np.empty((LAYERS, 2 * HID, HID), dtype=bf16)
    for l in range(LAYERS):
        w[l, :HID] = W_ih[l].T.astype(bf16)
        w[l, HID:] = W_hh[l].T.astype(bf16)
    bias = (b_ih.astype(np.float64) + b_hh.astype(np.float64)).astype(np.float32)
    # batch-major bias: partition 32g+b holds bias[g, :]
    bias_bm = np.repeat(bias, BC, axis=0).astype(np.float32)  # [128, H]
    ident = np.eye(128, dtype=bf16)

    shards = []
    for c in range(NCORES):
        grp, q = c // 4, c % 4
        t0 = 0 if grp == 0 else SEQ - TW
        xs = x[t0:t0 + TW, BC * q:BC * (q + 1), :]   # [TW, BC, H]
        xT = np.ascontiguousarray(xs.transpose(0, 2, 1))
        x_w = np.zeros((TW + XPAD, HID, BC), dtype=bf16)
        x_w[:TW] = xT.astype(bf16)
        shards.append({"x_w": x_w, "w_hi": w, "bias_bm": bias_bm,
                       "ident": ident})
    return shards


def kernel(x, W_ih, W_hh, b_ih, b_hh):
    x = np.asarray(x, dtype=np.float32)
    shards = _prep_inputs(x, np.asarray(W_ih), np.asarray(W_hh),
                          np.asarray(b_ih), np.asarray(b_hh))
    nc = build_kernel(repeat=1)
    res = run_bass_kernel_spmd(nc, shards, core_ids=list(range(NCORES)),
                               trace=False)
    out = np.empty((SEQ, BATCH, HID), dtype=np.float32)
    for c in range(NCORES):
        grp, q = c // 4, c % 4
        o = res.results[c]["out_bm"].astype(np.float32)   # [TW, BC, H]
        if grp == 0:
            out[0:TW, BC * q:BC * (q + 1), :] = o
        else:
            out[TW:SEQ, BC * q:BC * (q + 1), :] = o[WARM:]
    return out
